# revision 26
# baseline (speedup 1.0000x reference)
"""Trainium2 kernel for nn_CustomRNN (linear RNN, input_size=1, OUT=10).

Math: the RNN is linear:  h_t = h_{t-1} @ W2.T + x_t * w1,  y_t = h_t @ W3.T.
Unrolling:  y[b, t, :] = sum_{k>=0} x[b, t-k] * v_k,  v_k = W3 @ W2^k @ w1.
W2 ~ U(-0.05, 0.05) with H=256 has spectral radius ~0.48, so |v_k| decays by
~0.48 per step: |v_48|/|v_0| ~ 1e-16, far below f32 resolution.  Truncating to
K=64 taps is exact at f32 precision, turning the sequential scan into a short
causal FIR filter -> a handful of matmuls per batch.

Device scheme (per core, 8 batches):
  - Host pads each x row with K zeros and pre-transposes overlapping windows
    into slabs: slab_b[sigma, c] = x_pad[b, 64c + sigma], sigma,c in [0,128).
    Each slab is directly the matmul's stationary operand (no on-chip
    transpose needed).
  - Host also builds the banded FIR matrix A[sigma, tau*10+o] =
    v[tau+64-sigma, o]; inputs ship partition-major as in1=[A|slab0],
    in2=[slab1..slab7] -> two DMAs with 128 contiguous-row descriptors each.
  - Per batch: Y[c, (tau,o)] = slab.T @ A via two matmuls (N=512 + N=128,
    one PSUM bank each) in float32r (single-pass PE, ~4x fp32 rate; the PE
    rounds operands tf32-style, rel err ~1e-4 vs ~2e-7 for 2-pass fp32).
  - PSUM -> SBUF staging split DVE cols [0:384) / ACT [384:640); Y tile
    [128, 640] == y[b] (8192, 10) contiguous -> one 320KB DMA per batch,
    issues rotated over the sync/gpsimd/scalar sequencers.

Sharding: data-parallel over batch B=64 -> 8 batches/core on 8 cores;
A is replicated (320 KB).  Measured ~25.6us HW exec on trn2 (the platform
floor is ~13.5us: ~3.4us entry barrier + ~1.2us IRAM load + ~2.3us engine
prologue + tail barrier; output drain 2.6MB/core at ~264GB/s is the
dominant productive phase).
"""

import os

import numpy as np

B, T, H, OUT = 64, 8192, 256, 10
K = 64                     # FIR taps kept (tail is ~1e-21 relative)
NCORES = 8
BPC = B // NCORES          # batches per core
TP = K + T                 # padded time length (8256)
NCH = T // K               # 64-sample output chunks per batch (128)
AW = K * OUT               # A free width / Y row width (640)

_CACHE = {}

# test.py pokes this to request a traced run; results land in LAST_RESULTS.
TRACE = bool(os.environ.get("KERNEL_TRACE"))
TRACE_KWARGS = {}
LAST_RESULTS = None


def _build_program():
    import concourse.bass as bass
    import concourse.tile as tile
    from concourse import bacc, mybir
    from concourse.tile import add_dep_helper

    nc = bacc.Bacc(
        "TRN2",
        target_bir_lowering=False,
        debug=False,
        enable_asserts=False,
    )
    f32 = mybir.dt.float32
    # Partition-major merged input: in1 = [A | slab0] (128 x 768), in2 =
    # [slab1 .. slab7] (128 x 896); rows contiguous in HBM so each DMA is 128
    # descriptors of 3KB/3.5KB.  Tensors are declared float32r so the PE runs
    # single-pass reduced-precision matmuls on fp32 bits (walrus requires the
    # operand PRODUCER to be f32r-typed; a bitcast at the matmul is rejected).
    f32r = mybir.dt.float32r
    in1_d = nc.dram_tensor("in1", [128, AW + 128], f32r, kind="ExternalInput")
    in2_d = nc.dram_tensor("in2", [128, 3 * 128], f32r, kind="ExternalInput")
    in3_d = nc.dram_tensor("in3", [128, 4 * 128], f32r, kind="ExternalInput")
    y_d = nc.dram_tensor("y", [BPC, T, OUT], f32, kind="ExternalOutput")

    with tile.TileContext(nc) as tc:
        with (
            tc.tile_pool(name="consts", bufs=1) as consts,
            tc.tile_pool(name="ostage", bufs=8) as o_pool,
            tc.tile_pool(name="py", bufs=4, space="PSUM") as py_pool,
        ):
            # All three input DMAs on ONE sequencer, in1 first: per-queue
            # FIFO then guarantees in1 (which batch 0 needs) completes before
            # in2/in3 descriptors, instead of interleaving with them.
            in1_t = consts.tile([128, AW + 128], f32r)
            d1 = nc.sync.dma_start(out=in1_t, in_=in1_d.ap())
            in2_t = consts.tile([128, 3 * 128], f32r)
            d2 = nc.sync.dma_start(out=in2_t, in_=in2_d.ap())
            in3_t = consts.tile([128, 4 * 128], f32r)
            d3 = nc.gpsimd.dma_start(out=in3_t, in_=in3_d.ap())
            # in2/in3 start only after in1 completes, so batch 0's operands
            # get the full queue pool and land ~1.3us earlier.
            add_dep_helper(d2.ins, d1.ins, reason="in1 lands first")
            add_dep_helper(d3.ins, d1.ins, reason="in1 lands first")

            # Warm the scalar-engine activation table off the critical path.
            warm = consts.tile([1, 1], f32)
            nc.vector.memset(warm, 0.0)
            nc.scalar.copy(warm, warm)


            a0 = in1_t[:, 0:512]
            a1 = in1_t[:, 512:AW]
            for b in range(BPC):
                if b < 1:
                    lt = in1_t[:, AW:AW + 128]
                elif b < 4:
                    lt = in2_t[:, (b - 1) * 128:b * 128]
                else:
                    lt = in3_t[:, (b - 4) * 128:(b - 3) * 128]
                ps = py_pool.tile([128, AW], f32)
                nc.tensor.matmul(ps[:, 0:512], lt, a0, start=True, stop=True)
                nc.tensor.matmul(ps[:, 512:AW], lt, a1, start=True, stop=True)

                o = o_pool.tile([128, AW], f32)
                nc.vector.tensor_copy(o[:, 0:384], ps[:, 0:384])
                nc.scalar.copy(o[:, 384:AW], ps[:, 384:AW])

                dst = bass.AP(tensor=y_d, offset=b * T * OUT, ap=[[AW, 128], [1, AW]])
                eng = [nc.gpsimd, nc.scalar, nc.sync][b % 3]
                eng.dma_start(out=dst, in_=o)

    nc.compile()
    return nc


def _taps(W1, W2, W3):
    """v[k] = W3 @ W2^k @ w1 in float64, cast to f32."""
    w1 = np.asarray(W1, np.float64)[:, 0]
    W2d = np.asarray(W2, np.float64)
    W3d = np.asarray(W3, np.float64)
    v = np.zeros((K, OUT), np.float64)
    h = w1.copy()
    for k in range(K):
        v[k] = W3d @ h
        h = W2d @ h
    return v.astype(np.float32)


def _build_A(v):
    """A[sigma, tau*OUT + o] = v[tau + K - sigma, o] for 0 <= tau+K-sigma < K."""
    A = np.zeros((128, AW), np.float32)
    sig = np.arange(128)[:, None]
    tau = np.arange(K)[None, :]
    kk = tau + K - sig                      # [128, K]
    valid = (kk >= 0) & (kk < K)
    kk_c = np.clip(kk, 0, K - 1)
    Av = np.where(valid[:, :, None], v[kk_c], 0.0)   # [128, K, OUT]
    A[:, :] = Av.reshape(128, AW)
    return A


def kernel(x, W1, W2, W3):
    from concourse import bass_utils

    global LAST_RESULTS
    x = np.ascontiguousarray(np.asarray(x, np.float32))
    v = _taps(W1, W2, W3)
    A = _build_A(v)

    xpad = np.zeros((B, TP), np.float32)
    xpad[:, K:] = x
    # xw[b, sigma, c] = x_pad[b, 64c + sigma]
    xw = np.lib.stride_tricks.as_strided(
        xpad, shape=(B, 128, 128), strides=(TP * 4, 4, K * 4)
    )

    if "nc" not in _CACHE:
        _CACHE["nc"] = _build_program()
    nc = _CACHE["nc"]

    in_maps = []
    for i in range(NCORES):
        # [128, BPC*128] partition-major slab block for this core
        xwT = xw[i * BPC:(i + 1) * BPC].transpose(1, 0, 2).reshape(128, BPC * 128)
        in1 = np.concatenate([A, xwT[:, :128]], axis=1)
        in2 = np.ascontiguousarray(xwT[:, 128:512])
        in3 = np.ascontiguousarray(xwT[:, 512:])
        in_maps.append({"in1": in1, "in2": in2, "in3": in3})
    res = bass_utils.run_bass_kernel_spmd(
        nc,
        in_maps,
        core_ids=list(range(NCORES)),
        trace=TRACE,
        **TRACE_KWARGS,
    )
    LAST_RESULTS = res
    y = np.concatenate([res.results[i]["y"] for i in range(NCORES)], axis=0)
    return y.reshape(B, T, OUT)


# revision 27
# speedup vs baseline: 1.0744x; 1.0744x over previous
"""Trainium2 kernel for nn_CustomRNN (linear RNN, input_size=1, OUT=10).

Math: the RNN is linear:  h_t = h_{t-1} @ W2.T + x_t * w1,  y_t = h_t @ W3.T.
Unrolling:  y[b, t, :] = sum_{k>=0} x[b, t-k] * v_k,  v_k = W3 @ W2^k @ w1.
W2 ~ U(-0.05, 0.05) with H=256 has spectral radius ~0.48, so |v_k| decays by
~0.48 per step: |v_48|/|v_0| ~ 1e-16, far below f32 resolution.  Truncating to
K=64 taps is exact at f32 precision, turning the sequential scan into a short
causal FIR filter -> a handful of matmuls per batch.

Device scheme (per core, 8 batches):
  - Host pads each x row with K zeros and pre-transposes overlapping windows
    into slabs: slab_b[sigma, c] = x_pad[b, 64c + sigma], sigma,c in [0,128).
    Each slab is directly the matmul's stationary operand (no on-chip
    transpose needed).
  - Host also builds the banded FIR matrix A[sigma, tau*10+o] =
    v[tau+64-sigma, o]; inputs ship partition-major as in1=[A|slab0],
    in2=[slab1..slab7] -> two DMAs with 128 contiguous-row descriptors each.
  - Per batch: Y[c, (tau,o)] = slab.T @ A via two matmuls (N=512 + N=128,
    one PSUM bank each) in float32r (single-pass PE, ~4x fp32 rate; the PE
    rounds operands tf32-style, rel err ~1e-4 vs ~2e-7 for 2-pass fp32).
  - PSUM -> SBUF staging split DVE cols [0:384) / ACT [384:640); Y tile
    [128, 640] == y[b] (8192, 10) contiguous -> one 320KB DMA per batch,
    issues rotated over the sync/gpsimd/scalar sequencers.

Sharding: data-parallel over batch B=64 -> 8 batches/core on 8 cores;
A is replicated (320 KB).  Measured ~25.6us HW exec on trn2 (the platform
floor is ~13.5us: ~3.4us entry barrier + ~1.2us IRAM load + ~2.3us engine
prologue + tail barrier; output drain 2.6MB/core at ~264GB/s is the
dominant productive phase).
"""

import os

import numpy as np

B, T, H, OUT = 64, 8192, 256, 10
K = 64                     # FIR taps kept (tail is ~1e-21 relative)
NCORES = 8
BPC = B // NCORES          # batches per core
TP = K + T                 # padded time length (8256)
NCH = T // K               # 64-sample output chunks per batch (128)
AW = K * OUT               # A free width / Y row width (640)

_CACHE = {}

# test.py pokes this to request a traced run; results land in LAST_RESULTS.
TRACE = bool(os.environ.get("KERNEL_TRACE"))
TRACE_KWARGS = {}
LAST_RESULTS = None


def _build_program():
    import concourse.bass as bass
    import concourse.tile as tile
    from concourse import bacc, mybir

    nc = bacc.Bacc(
        "TRN2",
        target_bir_lowering=False,
        debug=False,
        enable_asserts=False,
    )
    f32 = mybir.dt.float32
    # Partition-major merged input: in1 = [A | slab0] (128 x 768), in2 =
    # [slab1 .. slab7] (128 x 896); rows contiguous in HBM so each DMA is 128
    # descriptors of 3KB/3.5KB.  Tensors are declared float32r so the PE runs
    # single-pass reduced-precision matmuls on fp32 bits (walrus requires the
    # operand PRODUCER to be f32r-typed; a bitcast at the matmul is rejected).
    f32r = mybir.dt.float32r
    in1_d = nc.dram_tensor("in1", [128, AW + 128], f32r, kind="ExternalInput")
    in2_d = nc.dram_tensor("in2", [128, 3 * 128], f32r, kind="ExternalInput")
    in3_d = nc.dram_tensor("in3", [128, 4 * 128], f32r, kind="ExternalInput")
    y_d = nc.dram_tensor("y", [BPC, T, OUT], f32, kind="ExternalOutput")

    with tile.TileContext(nc) as tc:
        with (
            tc.tile_pool(name="consts", bufs=1) as consts,
            tc.tile_pool(name="ostage", bufs=8) as o_pool,
            tc.tile_pool(name="py", bufs=4, space="PSUM") as py_pool,
        ):
            # All three input DMAs on ONE sequencer, in1 first: per-queue
            # FIFO then guarantees in1 (which batch 0 needs) completes before
            # in2/in3 descriptors, instead of interleaving with them.
            in1_t = consts.tile([128, AW + 128], f32r)
            nc.sync.dma_start(out=in1_t, in_=in1_d.ap())
            in2_t = consts.tile([128, 3 * 128], f32r)
            nc.sync.dma_start(out=in2_t, in_=in2_d.ap())
            in3_t = consts.tile([128, 4 * 128], f32r)
            nc.sync.dma_start(out=in3_t, in_=in3_d.ap())

            # Warm the scalar-engine activation table off the critical path.
            warm = consts.tile([1, 1], f32)
            nc.vector.memset(warm, 0.0)
            nc.scalar.copy(warm, warm)


            a0 = in1_t[:, 0:512]
            a1 = in1_t[:, 512:AW]
            for b in range(BPC):
                if b < 1:
                    lt = in1_t[:, AW:AW + 128]
                elif b < 4:
                    lt = in2_t[:, (b - 1) * 128:b * 128]
                else:
                    lt = in3_t[:, (b - 4) * 128:(b - 3) * 128]
                ps = py_pool.tile([128, AW], f32)
                nc.tensor.matmul(ps[:, 0:512], lt, a0, start=True, stop=True)
                nc.tensor.matmul(ps[:, 512:AW], lt, a1, start=True, stop=True)

                o = o_pool.tile([128, AW], f32)
                nc.vector.tensor_copy(o[:, 0:384], ps[:, 0:384])
                nc.scalar.copy(o[:, 384:AW], ps[:, 384:AW])

                dst = bass.AP(tensor=y_d, offset=b * T * OUT, ap=[[AW, 128], [1, AW]])
                eng = [nc.gpsimd, nc.scalar, nc.sync][b % 3]
                eng.dma_start(out=dst, in_=o)

    nc.compile()
    return nc


def _taps(W1, W2, W3):
    """v[k] = W3 @ W2^k @ w1 in float64, cast to f32."""
    w1 = np.asarray(W1, np.float64)[:, 0]
    W2d = np.asarray(W2, np.float64)
    W3d = np.asarray(W3, np.float64)
    v = np.zeros((K, OUT), np.float64)
    h = w1.copy()
    for k in range(K):
        v[k] = W3d @ h
        h = W2d @ h
    return v.astype(np.float32)


def _build_A(v):
    """A[sigma, tau*OUT + o] = v[tau + K - sigma, o] for 0 <= tau+K-sigma < K."""
    A = np.zeros((128, AW), np.float32)
    sig = np.arange(128)[:, None]
    tau = np.arange(K)[None, :]
    kk = tau + K - sig                      # [128, K]
    valid = (kk >= 0) & (kk < K)
    kk_c = np.clip(kk, 0, K - 1)
    Av = np.where(valid[:, :, None], v[kk_c], 0.0)   # [128, K, OUT]
    A[:, :] = Av.reshape(128, AW)
    return A


def kernel(x, W1, W2, W3):
    from concourse import bass_utils

    global LAST_RESULTS
    x = np.ascontiguousarray(np.asarray(x, np.float32))
    v = _taps(W1, W2, W3)
    A = _build_A(v)

    xpad = np.zeros((B, TP), np.float32)
    xpad[:, K:] = x
    # xw[b, sigma, c] = x_pad[b, 64c + sigma]
    xw = np.lib.stride_tricks.as_strided(
        xpad, shape=(B, 128, 128), strides=(TP * 4, 4, K * 4)
    )

    if "nc" not in _CACHE:
        _CACHE["nc"] = _build_program()
    nc = _CACHE["nc"]

    in_maps = []
    for i in range(NCORES):
        # [128, BPC*128] partition-major slab block for this core
        xwT = xw[i * BPC:(i + 1) * BPC].transpose(1, 0, 2).reshape(128, BPC * 128)
        in1 = np.concatenate([A, xwT[:, :128]], axis=1)
        in2 = np.ascontiguousarray(xwT[:, 128:512])
        in3 = np.ascontiguousarray(xwT[:, 512:])
        in_maps.append({"in1": in1, "in2": in2, "in3": in3})
    res = bass_utils.run_bass_kernel_spmd(
        nc,
        in_maps,
        core_ids=list(range(NCORES)),
        trace=TRACE,
        **TRACE_KWARGS,
    )
    LAST_RESULTS = res
    y = np.concatenate([res.results[i]["y"] for i in range(NCORES)], axis=0)
    return y.reshape(B, T, OUT)


# revision 30
# speedup vs baseline: 1.0954x; 1.0195x over previous
"""Trainium2 kernel for nn_CustomRNN (linear RNN, input_size=1, OUT=10).

Math: the RNN is linear:  h_t = h_{t-1} @ W2.T + x_t * w1,  y_t = h_t @ W3.T.
Unrolling:  y[b, t, :] = sum_{k>=0} x[b, t-k] * v_k,  v_k = W3 @ W2^k @ w1.
W2 ~ U(-0.05, 0.05) with H=256 has spectral radius ~0.48, so |v_k| decays by
~0.48 per step: |v_48|/|v_0| ~ 1e-16, far below f32 resolution.  Truncating to
K=64 taps is exact at f32 precision, turning the sequential scan into a short
causal FIR filter -> a handful of matmuls per batch.

Device scheme (per core, 8 batches):
  - Host pads each x row with K zeros and pre-transposes overlapping windows
    into slabs: slab_b[sigma, c] = x_pad[b, 64c + sigma], sigma,c in [0,128).
    Each slab is directly the matmul's stationary operand (no on-chip
    transpose needed).
  - Host also builds the banded FIR matrix A[sigma, tau*10+o] =
    v[tau+64-sigma, o]; inputs ship partition-major as in1=[A|slab0],
    in2=[slab1..3], in3=[slab4..7], all issued back-to-back from the sync
    sequencer (128 contiguous-row descriptors each).
  - Per batch: Y[c, (tau,o)] = slab.T @ A via two matmuls (N=512 + N=128,
    one PSUM bank each) in float32r (single-pass PE, ~4x fp32 rate; the PE
    rounds operands tf32-style, rel err ~1e-4 vs ~2e-7 for 2-pass fp32).
  - PSUM -> SBUF staging split DVE cols [0:384) / ACT [384:640); Y tile
    [128, 640] == y[b] (8192, 10) contiguous -> one 320KB DMA per batch,
    issues rotated over gpsimd/scalar/sync; the first two batches ship as
    two half-DMAs so the first bytes hit the then-idle queues right after
    the DVE copy.

Sharding: data-parallel over batch B=64 -> 8 batches/core on 8 cores;
A is replicated (320 KB).  Measured ~25.4us (median ~25.6us) HW exec on trn2 (the platform
floor is ~13.5us: ~3.4us entry barrier + ~1.2us IRAM load + ~2.3us engine
prologue + tail barrier; output drain 2.6MB/core at ~264GB/s is the
dominant productive phase).
"""

import os

import numpy as np

B, T, H, OUT = 64, 8192, 256, 10
K = 64                     # FIR taps kept (tail is ~1e-21 relative)
NCORES = 8
BPC = B // NCORES          # batches per core
TP = K + T                 # padded time length (8256)
NCH = T // K               # 64-sample output chunks per batch (128)
AW = K * OUT               # A free width / Y row width (640)

_CACHE = {}

# test.py pokes this to request a traced run; results land in LAST_RESULTS.
TRACE = bool(os.environ.get("KERNEL_TRACE"))
TRACE_KWARGS = {}
LAST_RESULTS = None


def _build_program():
    import concourse.bass as bass
    import concourse.tile as tile
    from concourse import bacc, mybir

    nc = bacc.Bacc(
        "TRN2",
        target_bir_lowering=False,
        debug=False,
        enable_asserts=False,
    )
    f32 = mybir.dt.float32
    # Partition-major merged input: in1 = [A | slab0] (128 x 768), in2 =
    # [slab1 .. slab7] (128 x 896); rows contiguous in HBM so each DMA is 128
    # descriptors of 3KB/3.5KB.  Tensors are declared float32r so the PE runs
    # single-pass reduced-precision matmuls on fp32 bits (walrus requires the
    # operand PRODUCER to be f32r-typed; a bitcast at the matmul is rejected).
    f32r = mybir.dt.float32r
    in1_d = nc.dram_tensor("in1", [128, AW + 128], f32r, kind="ExternalInput")
    in2_d = nc.dram_tensor("in2", [128, 3 * 128], f32r, kind="ExternalInput")
    in3_d = nc.dram_tensor("in3", [128, 4 * 128], f32r, kind="ExternalInput")
    y_d = nc.dram_tensor("y", [BPC, T, OUT], f32, kind="ExternalOutput")

    with tile.TileContext(nc) as tc:
        with (
            tc.tile_pool(name="consts", bufs=1) as consts,
            tc.tile_pool(name="ostage", bufs=8) as o_pool,
            tc.tile_pool(name="py", bufs=4, space="PSUM") as py_pool,
        ):
            # All three input DMAs on ONE sequencer, in1 first: per-queue
            # FIFO then guarantees in1 (which batch 0 needs) completes before
            # in2/in3 descriptors, instead of interleaving with them.
            in1_t = consts.tile([128, AW + 128], f32r)
            nc.sync.dma_start(out=in1_t, in_=in1_d.ap())
            in2_t = consts.tile([128, 3 * 128], f32r)
            nc.sync.dma_start(out=in2_t, in_=in2_d.ap())
            in3_t = consts.tile([128, 4 * 128], f32r)
            nc.sync.dma_start(out=in3_t, in_=in3_d.ap())

            # Warm the scalar-engine activation table off the critical path.
            warm = consts.tile([1, 1], f32)
            nc.vector.memset(warm, 0.0)
            nc.scalar.copy(warm, warm)


            a0 = in1_t[:, 0:512]
            a1 = in1_t[:, 512:AW]
            for b in range(BPC):
                if b < 1:
                    lt = in1_t[:, AW:AW + 128]
                elif b < 4:
                    lt = in2_t[:, (b - 1) * 128:b * 128]
                else:
                    lt = in3_t[:, (b - 4) * 128:(b - 3) * 128]
                ps = py_pool.tile([128, AW], f32)
                nc.tensor.matmul(ps[:, 0:512], lt, a0, start=True, stop=True)
                nc.tensor.matmul(ps[:, 512:AW], lt, a1, start=True, stop=True)

                o = o_pool.tile([128, AW], f32)
                nc.vector.tensor_copy(o[:, 0:384], ps[:, 0:384])
                nc.scalar.copy(o[:, 384:AW], ps[:, 384:AW])

                if b < 2:
                    # Queues are idle this early: split so the first half
                    # leaves right after the DVE copy, before ACT finishes.
                    dst0 = bass.AP(
                        tensor=y_d, offset=b * T * OUT, ap=[[AW, 128], [1, 384]]
                    )
                    dst1 = bass.AP(
                        tensor=y_d,
                        offset=b * T * OUT + 384,
                        ap=[[AW, 128], [1, AW - 384]],
                    )
                    nc.gpsimd.dma_start(out=dst0, in_=o[:, 0:384])
                    nc.sync.dma_start(out=dst1, in_=o[:, 384:AW])
                else:
                    dst = bass.AP(
                        tensor=y_d, offset=b * T * OUT, ap=[[AW, 128], [1, AW]]
                    )
                    eng = [nc.gpsimd, nc.scalar, nc.sync][b % 3]
                    eng.dma_start(out=dst, in_=o)

    nc.compile()
    return nc


def _taps(W1, W2, W3):
    """v[k] = W3 @ W2^k @ w1 in float64, cast to f32."""
    w1 = np.asarray(W1, np.float64)[:, 0]
    W2d = np.asarray(W2, np.float64)
    W3d = np.asarray(W3, np.float64)
    v = np.zeros((K, OUT), np.float64)
    h = w1.copy()
    for k in range(K):
        v[k] = W3d @ h
        h = W2d @ h
    return v.astype(np.float32)


def _build_A(v):
    """A[sigma, tau*OUT + o] = v[tau + K - sigma, o] for 0 <= tau+K-sigma < K."""
    A = np.zeros((128, AW), np.float32)
    sig = np.arange(128)[:, None]
    tau = np.arange(K)[None, :]
    kk = tau + K - sig                      # [128, K]
    valid = (kk >= 0) & (kk < K)
    kk_c = np.clip(kk, 0, K - 1)
    Av = np.where(valid[:, :, None], v[kk_c], 0.0)   # [128, K, OUT]
    A[:, :] = Av.reshape(128, AW)
    return A


def kernel(x, W1, W2, W3):
    from concourse import bass_utils

    global LAST_RESULTS
    x = np.ascontiguousarray(np.asarray(x, np.float32))
    v = _taps(W1, W2, W3)
    A = _build_A(v)

    xpad = np.zeros((B, TP), np.float32)
    xpad[:, K:] = x
    # xw[b, sigma, c] = x_pad[b, 64c + sigma]
    xw = np.lib.stride_tricks.as_strided(
        xpad, shape=(B, 128, 128), strides=(TP * 4, 4, K * 4)
    )

    if "nc" not in _CACHE:
        _CACHE["nc"] = _build_program()
    nc = _CACHE["nc"]

    in_maps = []
    for i in range(NCORES):
        # [128, BPC*128] partition-major slab block for this core
        xwT = xw[i * BPC:(i + 1) * BPC].transpose(1, 0, 2).reshape(128, BPC * 128)
        in1 = np.concatenate([A, xwT[:, :128]], axis=1)
        in2 = np.ascontiguousarray(xwT[:, 128:512])
        in3 = np.ascontiguousarray(xwT[:, 512:])
        in_maps.append({"in1": in1, "in2": in2, "in3": in3})
    res = bass_utils.run_bass_kernel_spmd(
        nc,
        in_maps,
        core_ids=list(range(NCORES)),
        trace=TRACE,
        **TRACE_KWARGS,
    )
    LAST_RESULTS = res
    y = np.concatenate([res.results[i]["y"] for i in range(NCORES)], axis=0)
    return y.reshape(B, T, OUT)
